# revision 5
# baseline (speedup 1.0000x reference)
"""Multi-head causal attention on 8 Trainium2 NeuronCores.

Problem: resid_pre [4, 2048, 1024], 16 heads x d_head 64, causal softmax,
output [4, 2048, 1024] f32.

Sharding: data-parallel over the 4 batches x tensor-parallel over 2 head
groups (8 heads each) -> 8 cores. Each core computes the attention output
contribution of its 8 heads for its batch; the host sums the two head-group
partials per batch (the "all-reduce") and adds the output bias.

Per-core kernel dataflow (all matmuls in float32r: full PE speed, ~1e-4 rel):
  phase V : V = X @ W_v for all 8 local heads, natural [seq, d] layout, with a
            ones-column appended per head (yields softmax denominators for
            free during the P@V matmul).
  then per head-pair p (heads 2p, 2p+1):
    QK  : Q^T, K^T = (X @ W_q)^T etc. via lhsT=W chunk, rhs=X^T chunk,
          pair-stacked on partitions (head 2p in partitions 0-63, 2p+1 in
          64-127) which makes the score matmuls row-tiled (concurrent).
    attn: per 512-wide query superblock, per 128-wide key tile:
          S^T = K^T.T @ Q^T (k on partitions) -> +mask on diagonal tiles ->
          exp on ScalarE (no max subtraction needed; scores are O(1)) ->
          z~^T[65,512] += V_chunk.T @ P~^T accumulated in PSUM (row 64 = sum
          of P~ = softmax denominator) -> normalize by reciprocal(denom).
  phase O : out[q,m] = sum_p z^T_p.T @ W_o_p, DMA PSUM -> DRAM.
"""

import numpy as np

import concourse.bass as bass
import concourse.mybir as mybir
import concourse.tile as tile
from concourse import bacc
from concourse import bass_utils

F32 = mybir.dt.float32
F32R = mybir.dt.float32r
EXPF = mybir.ActivationFunctionType.Exp

S = 2048          # sequence length
DM = 1024         # d_model
DH = 64           # d_head
NHC = 8           # heads per core
PAIRS = 4         # head pairs per core
MC = 8            # d_model chunks of 128
NSB = 4           # query superblocks of 512
SBW = 512         # superblock width
NKT = 16          # key tiles of 128
NST = 16          # seq tiles of 128
MASK_NEG = -1e9
SCALE = 0.125     # 1/sqrt(d_head)

_NC_CACHE = {}
LAST_RESULTS = None


def _build_nc():
    nc = bacc.Bacc("TRN2", target_bir_lowering=False, debug=False)

    xt_d = nc.dram_tensor("xt", [DM, S], F32R, kind="ExternalInput")
    wq_d = nc.dram_tensor("wq", [PAIRS, MC, 128, 128], F32R, kind="ExternalInput")
    wk_d = nc.dram_tensor("wk", [PAIRS, MC, 128, 128], F32R, kind="ExternalInput")
    wv_d = nc.dram_tensor("wv", [MC, 128, NHC * DH], F32R, kind="ExternalInput")
    wo_d = nc.dram_tensor("wo", [PAIRS, 128, DM], F32R, kind="ExternalInput")
    bq_d = nc.dram_tensor("bq", [PAIRS, 128, 1], F32, kind="ExternalInput")
    bk_d = nc.dram_tensor("bk", [PAIRS, 128, 1], F32, kind="ExternalInput")
    msk_d = nc.dram_tensor("mask", [4, 128, SBW], F32, kind="ExternalInput")
    out_d = nc.dram_tensor("out", [S, DM], F32, kind="ExternalOutput")

    with tile.TileContext(nc) as tc:
        with tc.tile_pool(name="hold", bufs=1) as hold:
            # persistent tiles (bufs=1 per tag)
            qt = hold.tile([128, S], F32R, tag="qt")
            kt = hold.tile([128, S], F32R, tag="kt")
            v_t = [hold.tile([128, NHC, DH + 1], F32R, tag=f"v{st}", name=f"v{st}") for st in range(NST)]
            z_t = [hold.tile([128, S], F32R, tag=f"z{p}", name=f"z{p}") for p in range(PAIRS)]
            msk_t = [hold.tile([128, SBW], F32, tag=f"m{i}", name=f"m{i}") for i in range(4)]
            bq_t = [hold.tile([128, 1], F32, tag=f"bq{p}", name=f"bq{p}") for p in range(PAIRS)]
            bk_t = [hold.tile([128, 1], F32, tag=f"bk{p}", name=f"bk{p}") for p in range(PAIRS)]
            ones_c = hold.tile([128, 1], F32, tag="ones")

            nc.vector.memset(ones_c[:], 1.0)
            for i in range(4):
                nc.sync.dma_start(msk_t[i][:], msk_d.ap()[i])
            for p in range(PAIRS):
                nc.sync.dma_start(bq_t[p][:], bq_d.ap()[p])
                nc.sync.dma_start(bk_t[p][:], bk_d.ap()[p])

            with (
                tc.tile_pool(name="ph12", bufs=1) as ph12,
                tc.tile_pool(name="pqk", bufs=1, space="PSUM") as pqk,
                tc.tile_pool(name="patn", bufs=1, space="PSUM") as patn,
            ):
                xt_t = [ph12.tile([128, S], F32R, tag=f"xt{m}", name=f"xt{m}") for m in range(MC)]
                for m in range(MC):
                    nc.sync.dma_start(xt_t[m][:], xt_d.ap()[m * 128:(m + 1) * 128, :])

                # ---- phase V: V (all 8 heads, natural layout) + ones column
                wv_t = [ph12.tile([128, NHC * DH], F32R, tag=f"wv{m}", name=f"wv{m}") for m in range(MC)]
                for m in range(MC):
                    nc.sync.dma_start(wv_t[m][:], wv_d.ap()[m])
                for st in range(NST):
                    ps = pqk.tile([128, 512], F32, tag="acc", bufs=2)
                    for m in range(MC):
                        nc.tensor.matmul(
                            ps[:],
                            xt_t[m][:, st * 128:(st + 1) * 128],
                            wv_t[m][:],
                            start=(m == 0),
                            stop=(m == MC - 1),
                        )
                    nc.vector.tensor_copy(
                        v_t[st][:, :, 0:DH],
                        ps[:].rearrange("p (h d) -> p h d", h=NHC),
                    )
                    nc.vector.tensor_copy(
                        v_t[st][:, :, DH],
                        ones_c[:].to_broadcast((128, NHC)),
                    )

                # ---- per head pair: QK projection then attention
                for p in range(PAIRS):
                    for (w_d, b_t, dst) in ((wq_d, bq_t, qt), (wk_d, bk_t, kt)):
                        wts = []
                        for m in range(MC):
                            w = ph12.tile([128, 128], F32R, tag="w", bufs=10)
                            nc.sync.dma_start(w[:], w_d.ap()[p, m])
                            wts.append(w)
                        for sb in range(NSB):
                            ps = pqk.tile([128, 512], F32, tag="acc", bufs=2)
                            for m in range(MC):
                                nc.tensor.matmul(
                                    ps[:],
                                    wts[m][:],
                                    xt_t[m][:, sb * SBW:(sb + 1) * SBW],
                                    start=(m == 0),
                                    stop=(m == MC - 1),
                                )
                            nc.vector.tensor_scalar_add(
                                dst[:, sb * SBW:(sb + 1) * SBW], ps[:], b_t[p][:]
                            )

                    # attention for heads (2p, 2p+1)
                    for sb in range(NSB):
                        nkt = 4 * (sb + 1)
                        z0 = patn.tile([DH + 1, 512], F32, tag="z0", bufs=1)
                        z1 = patn.tile([DH + 1, 512], F32, tag="z1", bufs=1)
                        qs = (sb * SBW, (sb + 1) * SBW)
                        for j in range(nkt):
                            sp = patn.tile([128, 1024], F32, tag="sp", bufs=2)
                            ks = (j * 128, (j + 1) * 128)
                            nc.tensor.matmul(
                                sp[:, 0:512],
                                kt[0:64, ks[0]:ks[1]],
                                qt[0:64, qs[0]:qs[1]],
                                start=True, stop=True,
                                tile_position=(0, 0),
                            )
                            nc.tensor.matmul(
                                sp[:, 512:1024],
                                kt[64:128, ks[0]:ks[1]],
                                qt[64:128, qs[0]:qs[1]],
                                start=True, stop=True,
                                tile_position=(64, 0),
                            )
                            j_rel = j - 4 * sb
                            if j_rel >= 0:
                                nc.vector.tensor_add(
                                    sp[:, 0:512], sp[:, 0:512], msk_t[j_rel][:]
                                )
                                nc.vector.tensor_add(
                                    sp[:, 512:1024], sp[:, 512:1024], msk_t[j_rel][:]
                                )
                            pt = ph12.tile([128, 1024], F32R, tag="pt", bufs=3)
                            nc.scalar.activation(pt[:], sp[:], EXPF, scale=SCALE)
                            nc.tensor.matmul(
                                z0[:],
                                v_t[j][:, 2 * p, :],
                                pt[:, 0:512],
                                start=(j == 0), stop=(j == nkt - 1),
                            )
                            nc.tensor.matmul(
                                z1[:],
                                v_t[j][:, 2 * p + 1, :],
                                pt[:, 512:1024],
                                start=(j == 0), stop=(j == nkt - 1),
                            )
                        # normalize by softmax denominator (row DH of z psum):
                        # pull the denom row to SBUF (same partition), invert it
                        # in place, broadcast-DMA across 64 partitions, multiply.
                        d0row = ph12.tile([1, 512], F32, tag="d0row", bufs=2)
                        d1row = ph12.tile([1, 512], F32, tag="d1row", bufs=2)
                        nc.vector.reciprocal(d0row[:], z0[DH:DH + 1, :])
                        nc.vector.reciprocal(d1row[:], z1[DH:DH + 1, :])
                        r0 = ph12.tile([64, 512], F32, tag="r0", bufs=2)
                        r1 = ph12.tile([64, 512], F32, tag="r1", bufs=2)
                        nc.gpsimd.partition_broadcast(r0[:], d0row[:], channels=64)
                        nc.gpsimd.partition_broadcast(r1[:], d1row[:], channels=64)
                        nc.vector.tensor_mul(
                            z_t[p][0:64, qs[0]:qs[1]], z0[0:64, :], r0[:]
                        )
                        t1 = ph12.tile([64, 512], F32R, tag="t1", bufs=2)
                        nc.vector.tensor_mul(t1[:], z1[0:64, :], r1[:])
                        nc.sync.dma_start(z_t[p][64:128, qs[0]:qs[1]], t1[:])

            # ---- phase O: out[q, m] = sum_p z_p^T.T @ wo_p
            with (
                tc.tile_pool(name="ph3", bufs=1) as ph3,
                tc.tile_pool(name="po", bufs=1, space="PSUM") as po,
            ):
                wo_t = [ph3.tile([128, DM], F32R, tag=f"wo{p}", name=f"wo{p}") for p in range(PAIRS)]
                for p in range(PAIRS):
                    nc.sync.dma_start(wo_t[p][:], wo_d.ap()[p])
                for q in range(NST):
                    for mb in range(2):
                        ps = po.tile([128, 512], F32, tag="o", bufs=4)
                        for p in range(PAIRS):
                            nc.tensor.matmul(
                                ps[:],
                                z_t[p][:, q * 128:(q + 1) * 128],
                                wo_t[p][:, mb * 512:(mb + 1) * 512],
                                start=(p == 0),
                                stop=(p == PAIRS - 1),
                            )
                        ost = ph3.tile([128, 512], F32, tag="ost", bufs=4)
                        nc.vector.tensor_copy(ost[:], ps[:])
                        nc.sync.dma_start(
                            out_d.ap()[q * 128:(q + 1) * 128, mb * 512:(mb + 1) * 512],
                            ost[:],
                        )

    nc.compile()
    return nc


def _get_nc():
    if "nc" not in _NC_CACHE:
        _NC_CACHE["nc"] = _build_nc()
    return _NC_CACHE["nc"]


def _causal_masks():
    k = np.arange(128)[:, None]
    q = np.arange(SBW)[None, :]
    return np.stack(
        [np.where(q >= j * 128 + k, 0.0, MASK_NEG) for j in range(4)]
    ).astype(np.float32)


def kernel(resid_pre, W_Q, W_K, W_V, W_O, b_Q, b_K, b_V, b_O):
    global LAST_RESULTS
    resid_pre = np.asarray(resid_pre, dtype=np.float32)
    W_Q = np.asarray(W_Q, dtype=np.float32)
    W_K = np.asarray(W_K, dtype=np.float32)
    W_V = np.asarray(W_V, dtype=np.float32)
    W_O = np.asarray(W_O, dtype=np.float32)
    b_Q = np.asarray(b_Q, dtype=np.float32)
    b_K = np.asarray(b_K, dtype=np.float32)
    b_V = np.asarray(b_V, dtype=np.float32)
    b_O = np.asarray(b_O, dtype=np.float32)

    B = resid_pre.shape[0]
    masks = _causal_masks()

    def pack_pairs(w):  # [8, 1024, 64] -> [4, 8, 128, 128]
        return np.ascontiguousarray(
            w.reshape(PAIRS, 2, DM, DH).transpose(0, 2, 1, 3).reshape(PAIRS, MC, 128, 128)
        )

    in_maps = []
    for c in range(8):
        b, g = divmod(c, 2)
        hs = slice(g * NHC, (g + 1) * NHC)
        in_maps.append({
            "xt": np.ascontiguousarray(resid_pre[b].T),
            "wq": pack_pairs(W_Q[hs]),
            "wk": pack_pairs(W_K[hs]),
            "wv": np.ascontiguousarray(
                W_V[hs].transpose(1, 0, 2).reshape(DM, NHC * DH).reshape(MC, 128, NHC * DH)
            ),
            "wo": np.ascontiguousarray(W_O[hs].reshape(PAIRS, 128, DM)),
            "bq": np.ascontiguousarray(b_Q[hs].reshape(PAIRS, 128, 1)),
            "bk": np.ascontiguousarray(b_K[hs].reshape(PAIRS, 128, 1)),
            "mask": masks,
        })

    nc = _get_nc()
    res = bass_utils.run_bass_kernel_spmd(nc, in_maps, core_ids=list(range(8)))
    LAST_RESULTS = res

    # b_V contributes exactly sum_h W_O[h].T @ b_V[h] (softmax rows sum to 1)
    const = np.einsum("hdm,hd->m", W_O, b_V).astype(np.float32) + b_O
    out = np.empty((B, S, DM), dtype=np.float32)
    for b in range(B):
        out[b] = res.results[2 * b]["out"] + res.results[2 * b + 1]["out"] + const
    return out


# revision 7
# speedup vs baseline: 1.2495x; 1.2495x over previous
"""Multi-head causal attention on 8 Trainium2 NeuronCores.

Problem: resid_pre [4, 2048, 1024], 16 heads x d_head 64, causal softmax,
output [4, 2048, 1024] f32.

Sharding: data-parallel over the 4 batches x tensor-parallel over 2 head
groups (8 heads each) -> 8 cores. Each core computes the attention output
contribution of its 8 heads for its batch; the host sums the two head-group
partials per batch (the "all-reduce") and adds the output bias.

Per-core kernel dataflow (all matmuls in float32r: full PE speed, ~1e-4 rel):
  phase V : V = X @ W_v for all 8 local heads, natural [seq, d] layout, with a
            ones-column appended per head (yields softmax denominators for
            free during the P@V matmul).
  then per head-pair p (heads 2p, 2p+1):
    QK  : Q^T, K^T = (X @ W_q)^T etc. via lhsT=W chunk, rhs=X^T chunk,
          pair-stacked on partitions (head 2p in partitions 0-63, 2p+1 in
          64-127) which makes the score matmuls row-tiled (concurrent).
    attn: per 512-wide query superblock, per 128-wide key tile:
          S^T = K^T.T @ Q^T (k on partitions) -> +mask on diagonal tiles ->
          exp on ScalarE (no max subtraction needed; scores are O(1)) ->
          z~^T[65,512] += V_chunk.T @ P~^T accumulated in PSUM (row 64 = sum
          of P~ = softmax denominator) -> normalize by reciprocal(denom).
  phase O : out[q,m] = sum_p z^T_p.T @ W_o_p, DMA PSUM -> DRAM.
"""

import numpy as np

import concourse.bass as bass
import concourse.mybir as mybir
import concourse.tile as tile
from concourse import bacc
from concourse import bass_utils

F32 = mybir.dt.float32
F32R = mybir.dt.float32r
EXPF = mybir.ActivationFunctionType.Exp

S = 2048          # sequence length
DM = 1024         # d_model
DH = 64           # d_head
NHC = 8           # heads per core
PAIRS = 4         # head pairs per core
MC = 8            # d_model chunks of 128
NSB = 4           # query superblocks of 512
SBW = 512         # superblock width
NKT = 16          # key tiles of 128
NST = 16          # seq tiles of 128
MASK_NEG = -1e9
SCALE = 0.125     # 1/sqrt(d_head)

_NC_CACHE = {}
LAST_RESULTS = None


def _build_nc():
    nc = bacc.Bacc("TRN2", target_bir_lowering=False, debug=False)

    xt_d = nc.dram_tensor("xt", [DM, S], F32R, kind="ExternalInput")
    wq_d = nc.dram_tensor("wq", [PAIRS, MC, 128, 128], F32R, kind="ExternalInput")
    wk_d = nc.dram_tensor("wk", [PAIRS, MC, 128, 128], F32R, kind="ExternalInput")
    wv_d = nc.dram_tensor("wv", [MC, 128, NHC * DH], F32R, kind="ExternalInput")
    wo_d = nc.dram_tensor("wo", [PAIRS, 128, DM], F32R, kind="ExternalInput")
    bq_d = nc.dram_tensor("bq", [PAIRS, 128, 1], F32, kind="ExternalInput")
    bk_d = nc.dram_tensor("bk", [PAIRS, 128, 1], F32, kind="ExternalInput")
    msk_d = nc.dram_tensor("mask", [4, 128, SBW], F32, kind="ExternalInput")
    out_d = nc.dram_tensor("out", [S, DM], F32, kind="ExternalOutput")

    with tile.TileContext(nc) as tc:
        with tc.tile_pool(name="hold", bufs=1) as hold:
            # persistent tiles (bufs=1 per tag)
            qt = hold.tile([128, S], F32R, tag="qt")
            kt = hold.tile([128, S], F32R, tag="kt")
            v_t = [hold.tile([128, NHC, DH + 1], F32R, tag=f"v{st}", name=f"v{st}") for st in range(NST)]
            z_t = [hold.tile([128, S], F32R, tag=f"z{p}", name=f"z{p}") for p in range(PAIRS)]
            msk_t = [hold.tile([128, SBW], F32, tag=f"m{i}", name=f"m{i}") for i in range(4)]
            bq_t = [hold.tile([128, 1], F32, tag=f"bq{p}", name=f"bq{p}") for p in range(PAIRS)]
            bk_t = [hold.tile([128, 1], F32, tag=f"bk{p}", name=f"bk{p}") for p in range(PAIRS)]
            ones_c = hold.tile([128, 1], F32, tag="ones")

            nc.vector.memset(ones_c[:], 1.0)
            for i in range(4):
                nc.sync.dma_start(msk_t[i][:], msk_d.ap()[i])
            for p in range(PAIRS):
                nc.sync.dma_start(bq_t[p][:], bq_d.ap()[p])
                nc.sync.dma_start(bk_t[p][:], bk_d.ap()[p])

            with (
                tc.tile_pool(name="ph12", bufs=1) as ph12,
                tc.tile_pool(name="pqk", bufs=1, space="PSUM") as pqk,
                tc.tile_pool(name="patn", bufs=1, space="PSUM") as patn,
            ):
                xt_t = [ph12.tile([128, S], F32R, tag=f"xt{m}", name=f"xt{m}") for m in range(MC)]
                wv_t = [ph12.tile([128, NHC * DH], F32R, tag=f"wv{m}", name=f"wv{m}") for m in range(MC)]
                # interleave xt/wv loads so the V-projection m-loop can chase the DMAs
                for m in range(MC):
                    nc.sync.dma_start(xt_t[m][:], xt_d.ap()[m * 128:(m + 1) * 128, :])
                    nc.sync.dma_start(wv_t[m][:], wv_d.ap()[m])
                for st in range(NST):
                    ps = pqk.tile([128, 512], F32, tag="acc", bufs=2)
                    for m in range(MC):
                        nc.tensor.matmul(
                            ps[:],
                            xt_t[m][:, st * 128:(st + 1) * 128],
                            wv_t[m][:],
                            start=(m == 0),
                            stop=(m == MC - 1),
                        )
                    nc.vector.tensor_copy(
                        v_t[st][:, :, 0:DH],
                        ps[:].rearrange("p (h d) -> p h d", h=NHC),
                    )
                    nc.vector.tensor_copy(
                        v_t[st][:, :, DH],
                        ones_c[:].to_broadcast((128, NHC)),
                    )

                # ---- per head pair: QK projection then attention
                for p in range(PAIRS):
                    for (w_d, b_t, dst) in ((wq_d, bq_t, qt), (wk_d, bk_t, kt)):
                        wts = []
                        for m in range(MC):
                            w = ph12.tile([128, 128], F32R, tag="w", bufs=10)
                            nc.sync.dma_start(w[:], w_d.ap()[p, m])
                            wts.append(w)
                        for sb in range(NSB):
                            ps = pqk.tile([128, 512], F32, tag="acc", bufs=2)
                            for m in range(MC):
                                nc.tensor.matmul(
                                    ps[:],
                                    wts[m][:],
                                    xt_t[m][:, sb * SBW:(sb + 1) * SBW],
                                    start=(m == 0),
                                    stop=(m == MC - 1),
                                )
                            nc.vector.tensor_scalar_add(
                                dst[:, sb * SBW:(sb + 1) * SBW], ps[:], b_t[p][:]
                            )

                    # attention for heads (2p, 2p+1)
                    for sb in range(NSB):
                        nkt = 4 * (sb + 1)
                        z0 = patn.tile([DH + 1, 512], F32, tag="z0", bufs=1)
                        z1 = patn.tile([DH + 1, 512], F32, tag="z1", bufs=1)
                        qs = (sb * SBW, (sb + 1) * SBW)
                        for j in range(nkt):
                            sp = patn.tile([128, 1024], F32, tag="sp", bufs=2)
                            ks = (j * 128, (j + 1) * 128)
                            nc.tensor.matmul(
                                sp[:, 0:512],
                                kt[0:64, ks[0]:ks[1]],
                                qt[0:64, qs[0]:qs[1]],
                                start=True, stop=True,
                                tile_position=(0, 0),
                            )
                            nc.tensor.matmul(
                                sp[:, 512:1024],
                                kt[64:128, ks[0]:ks[1]],
                                qt[64:128, qs[0]:qs[1]],
                                start=True, stop=True,
                                tile_position=(64, 0),
                            )
                            j_rel = j - 4 * sb
                            if j_rel >= 0:
                                nc.vector.tensor_add(
                                    sp[:, 0:512], sp[:, 0:512], msk_t[j_rel][:]
                                )
                                nc.vector.tensor_add(
                                    sp[:, 512:1024], sp[:, 512:1024], msk_t[j_rel][:]
                                )
                            pt = ph12.tile([128, 1024], F32R, tag="pt", bufs=3)
                            nc.scalar.activation(pt[:], sp[:], EXPF, scale=SCALE)
                            nc.tensor.matmul(
                                z0[:],
                                v_t[j][:, 2 * p, :],
                                pt[:, 0:512],
                                start=(j == 0), stop=(j == nkt - 1),
                            )
                            nc.tensor.matmul(
                                z1[:],
                                v_t[j][:, 2 * p + 1, :],
                                pt[:, 512:1024],
                                start=(j == 0), stop=(j == nkt - 1),
                            )
                        # normalize by softmax denominator (row DH of z psum):
                        # pull the denom row to SBUF (same partition), invert it
                        # in place, broadcast-DMA across 64 partitions, multiply.
                        d0row = ph12.tile([1, 512], F32, tag="d0row", bufs=2)
                        d1row = ph12.tile([1, 512], F32, tag="d1row", bufs=2)
                        nc.vector.tensor_copy(d0row[:], z0[DH:DH + 1, :])
                        nc.vector.tensor_copy(d1row[:], z1[DH:DH + 1, :])
                        nc.vector.reciprocal_approx_fast(d0row[:], d0row[:])
                        nc.vector.reciprocal_approx_fast(d1row[:], d1row[:])
                        r0 = ph12.tile([64, 512], F32, tag="r0", bufs=2)
                        r1 = ph12.tile([64, 512], F32, tag="r1", bufs=2)
                        nc.gpsimd.partition_broadcast(r0[:], d0row[:], channels=64)
                        nc.gpsimd.partition_broadcast(r1[:], d1row[:], channels=64)
                        nc.vector.tensor_mul(
                            z_t[p][0:64, qs[0]:qs[1]], z0[0:64, :], r0[:]
                        )
                        t1 = ph12.tile([64, 512], F32R, tag="t1", bufs=2)
                        nc.vector.tensor_mul(t1[:], z1[0:64, :], r1[:])
                        nc.sync.dma_start(z_t[p][64:128, qs[0]:qs[1]], t1[:])

            # ---- phase O: out[q, m] = sum_p z_p^T.T @ wo_p
            with (
                tc.tile_pool(name="ph3", bufs=1) as ph3,
                tc.tile_pool(name="po", bufs=1, space="PSUM") as po,
            ):
                wo_t = [ph3.tile([128, DM], F32R, tag=f"wo{p}", name=f"wo{p}") for p in range(PAIRS)]
                for p in range(PAIRS):
                    nc.sync.dma_start(wo_t[p][:], wo_d.ap()[p])
                for q in range(NST):
                    for mb in range(2):
                        ps = po.tile([128, 512], F32, tag="o", bufs=4)
                        for p in range(PAIRS):
                            nc.tensor.matmul(
                                ps[:],
                                z_t[p][:, q * 128:(q + 1) * 128],
                                wo_t[p][:, mb * 512:(mb + 1) * 512],
                                start=(p == 0),
                                stop=(p == PAIRS - 1),
                            )
                        ost = ph3.tile([128, 512], F32, tag="ost", bufs=4)
                        nc.vector.tensor_copy(ost[:], ps[:])
                        nc.sync.dma_start(
                            out_d.ap()[q * 128:(q + 1) * 128, mb * 512:(mb + 1) * 512],
                            ost[:],
                        )

    nc.compile()
    return nc


def _get_nc():
    if "nc" not in _NC_CACHE:
        _NC_CACHE["nc"] = _build_nc()
    return _NC_CACHE["nc"]


def _causal_masks():
    k = np.arange(128)[:, None]
    q = np.arange(SBW)[None, :]
    return np.stack(
        [np.where(q >= j * 128 + k, 0.0, MASK_NEG) for j in range(4)]
    ).astype(np.float32)


def kernel(resid_pre, W_Q, W_K, W_V, W_O, b_Q, b_K, b_V, b_O):
    global LAST_RESULTS
    resid_pre = np.asarray(resid_pre, dtype=np.float32)
    W_Q = np.asarray(W_Q, dtype=np.float32)
    W_K = np.asarray(W_K, dtype=np.float32)
    W_V = np.asarray(W_V, dtype=np.float32)
    W_O = np.asarray(W_O, dtype=np.float32)
    b_Q = np.asarray(b_Q, dtype=np.float32)
    b_K = np.asarray(b_K, dtype=np.float32)
    b_V = np.asarray(b_V, dtype=np.float32)
    b_O = np.asarray(b_O, dtype=np.float32)

    B = resid_pre.shape[0]
    masks = _causal_masks()

    def pack_pairs(w):  # [8, 1024, 64] -> [4, 8, 128, 128]
        return np.ascontiguousarray(
            w.reshape(PAIRS, 2, DM, DH).transpose(0, 2, 1, 3).reshape(PAIRS, MC, 128, 128)
        )

    in_maps = []
    for c in range(8):
        b, g = divmod(c, 2)
        hs = slice(g * NHC, (g + 1) * NHC)
        in_maps.append({
            "xt": np.ascontiguousarray(resid_pre[b].T),
            "wq": pack_pairs(W_Q[hs]),
            "wk": pack_pairs(W_K[hs]),
            "wv": np.ascontiguousarray(
                W_V[hs].transpose(1, 0, 2).reshape(DM, NHC * DH).reshape(MC, 128, NHC * DH)
            ),
            "wo": np.ascontiguousarray(W_O[hs].reshape(PAIRS, 128, DM)),
            "bq": np.ascontiguousarray(b_Q[hs].reshape(PAIRS, 128, 1)),
            "bk": np.ascontiguousarray(b_K[hs].reshape(PAIRS, 128, 1)),
            "mask": masks,
        })

    nc = _get_nc()
    res = bass_utils.run_bass_kernel_spmd(nc, in_maps, core_ids=list(range(8)))
    LAST_RESULTS = res

    # b_V contributes exactly sum_h W_O[h].T @ b_V[h] (softmax rows sum to 1)
    const = np.einsum("hdm,hd->m", W_O, b_V).astype(np.float32) + b_O
    out = np.empty((B, S, DM), dtype=np.float32)
    for b in range(B):
        out[b] = res.results[2 * b]["out"] + res.results[2 * b + 1]["out"] + const
    return out


# revision 11
# speedup vs baseline: 1.2556x; 1.0048x over previous
"""Multi-head causal attention on 8 Trainium2 NeuronCores.

Problem: resid_pre [4, 2048, 1024], 16 heads x d_head 64, causal softmax,
output [4, 2048, 1024] f32.

Sharding: data-parallel over the 4 batches x tensor-parallel over 2 head
groups (8 heads each) -> 8 cores. Each core computes the attention output
contribution of its 8 heads for its batch; the host sums the two head-group
partials per batch (the "all-reduce") and adds the output bias.

Per-core kernel dataflow (all matmuls in float32r: full PE speed, ~1e-4 rel):
  phase V : V = X @ W_v for all 8 local heads, natural [seq, d] layout, with a
            ones-column appended per head (yields softmax denominators for
            free during the P@V matmul).
  then per head-pair p (heads 2p, 2p+1):
    QK  : Q^T, K^T = (X @ W_q)^T etc. via lhsT=W chunk, rhs=X^T chunk,
          pair-stacked on partitions (head 2p in partitions 0-63, 2p+1 in
          64-127) which makes the score matmuls row-tiled (concurrent).
    attn: per 512-wide query superblock, per 128-wide key tile:
          S^T = K^T.T @ Q^T (k on partitions) -> +mask on diagonal tiles ->
          exp on ScalarE (no max subtraction needed; scores are O(1)) ->
          z~^T[65,512] += V_chunk.T @ P~^T accumulated in PSUM (row 64 = sum
          of P~ = softmax denominator) -> normalize by reciprocal(denom).
  phase O : out[q,m] = sum_p z^T_p.T @ W_o_p, DMA PSUM -> DRAM.
"""

import numpy as np

import concourse.bass as bass
import concourse.mybir as mybir
import concourse.tile as tile
from concourse import bacc
from concourse import bass_utils

F32 = mybir.dt.float32
F32R = mybir.dt.float32r
EXPF = mybir.ActivationFunctionType.Exp

S = 2048          # sequence length
DM = 1024         # d_model
DH = 64           # d_head
NHC = 8           # heads per core
PAIRS = 4         # head pairs per core
MC = 8            # d_model chunks of 128
NSB = 4           # query superblocks of 512
SBW = 512         # superblock width
NKT = 16          # key tiles of 128
NST = 16          # seq tiles of 128
MASK_NEG = -1e9
SCALE = 0.125     # 1/sqrt(d_head)

_NC_CACHE = {}
LAST_RESULTS = None


def _build_nc():
    nc = bacc.Bacc("TRN2", target_bir_lowering=False, debug=False)

    xt_d = nc.dram_tensor("xt", [DM, S], F32R, kind="ExternalInput")
    wq_d = nc.dram_tensor("wq", [PAIRS, MC, 128, 128], F32R, kind="ExternalInput")
    wk_d = nc.dram_tensor("wk", [PAIRS, MC, 128, 128], F32R, kind="ExternalInput")
    wv_d = nc.dram_tensor("wv", [MC, 128, NHC * DH], F32R, kind="ExternalInput")
    wo_d = nc.dram_tensor("wo", [PAIRS, 128, DM], F32R, kind="ExternalInput")
    bq_d = nc.dram_tensor("bq", [PAIRS, 128, 1], F32, kind="ExternalInput")
    bk_d = nc.dram_tensor("bk", [PAIRS, 128, 1], F32, kind="ExternalInput")
    msk_d = nc.dram_tensor("mask", [128, 128], F32, kind="ExternalInput")
    out_d = nc.dram_tensor("out", [S, DM], F32, kind="ExternalOutput")

    with tile.TileContext(nc) as tc:
        with tc.tile_pool(name="hold", bufs=1) as hold:
            # persistent tiles (bufs=1 per tag)
            # NB: allocated per pair inside the loop (bufs=2) so next pair's
            # projection overlaps this pair's attention.
            v_t = [hold.tile([128, NHC, DH + 1], F32R, tag=f"v{st}", name=f"v{st}") for st in range(NST)]
            z_t = [hold.tile([128, S], F32R, tag=f"z{p}", name=f"z{p}") for p in range(PAIRS)]
            msk_t = hold.tile([128, 128], F32, tag="mtri")
            bq_t = [hold.tile([128, 1], F32, tag=f"bq{p}", name=f"bq{p}") for p in range(PAIRS)]
            bk_t = [hold.tile([128, 1], F32, tag=f"bk{p}", name=f"bk{p}") for p in range(PAIRS)]
            ones_c = hold.tile([128, 1], F32, tag="ones")

            nc.vector.memset(ones_c[:], 1.0)
            nc.sync.dma_start(msk_t[:], msk_d.ap())
            for p in range(PAIRS):
                nc.sync.dma_start(bq_t[p][:], bq_d.ap()[p])
                nc.sync.dma_start(bk_t[p][:], bk_d.ap()[p])

            with (
                tc.tile_pool(name="ph12", bufs=1) as ph12,
                tc.tile_pool(name="pqk", bufs=1, space="PSUM") as pqk,
                tc.tile_pool(name="patn", bufs=1, space="PSUM") as patn,
            ):
                xt_t = [ph12.tile([128, S], F32R, tag=f"xt{m}", name=f"xt{m}") for m in range(MC)]
                wv_t = [ph12.tile([128, NHC * DH], F32R, tag=f"wv{m}", name=f"wv{m}") for m in range(MC)]
                # interleave xt/wv loads so the V-projection m-loop can chase the DMAs
                for m in range(MC):
                    nc.sync.dma_start(xt_t[m][:], xt_d.ap()[m * 128:(m + 1) * 128, :])
                    nc.sync.dma_start(wv_t[m][:], wv_d.ap()[m])
                for st in range(NST):
                    ps = pqk.tile([128, 512], F32, tag="acc", bufs=2)
                    for m in range(MC):
                        nc.tensor.matmul(
                            ps[:],
                            xt_t[m][:, st * 128:(st + 1) * 128],
                            wv_t[m][:],
                            start=(m == 0),
                            stop=(m == MC - 1),
                        )
                    nc.vector.tensor_copy(
                        v_t[st][:, :, 0:DH],
                        ps[:].rearrange("p (h d) -> p h d", h=NHC),
                    )
                    nc.vector.tensor_copy(
                        v_t[st][:, :, DH],
                        ones_c[:].to_broadcast((128, NHC)),
                    )

                # ---- per head pair: QK projection then attention
                for p in range(PAIRS):
                    qt = ph12.tile([128, S], F32R, tag="qt", bufs=2, name="qt")
                    kt = ph12.tile([128, S], F32R, tag="kt", bufs=2, name="kt")
                    for (w_d, b_t, dst) in ((wq_d, bq_t, qt), (wk_d, bk_t, kt)):
                        wts = []
                        for m in range(MC):
                            w = ph12.tile([128, 128], F32R, tag="w", bufs=8)
                            nc.sync.dma_start(w[:], w_d.ap()[p, m])
                            wts.append(w)
                        for sb in range(NSB):
                            ps = pqk.tile([128, 512], F32, tag="acc", bufs=2)
                            for m in range(MC):
                                nc.tensor.matmul(
                                    ps[:],
                                    wts[m][:],
                                    xt_t[m][:, sb * SBW:(sb + 1) * SBW],
                                    start=(m == 0),
                                    stop=(m == MC - 1),
                                )
                            nc.vector.tensor_scalar_add(
                                dst[:, sb * SBW:(sb + 1) * SBW], ps[:], b_t[p][:]
                            )

                    # attention for heads (2p, 2p+1)
                    for sb in range(NSB):
                        nkt = 4 * (sb + 1)
                        z0 = patn.tile([DH + 1, 512], F32, tag="z0", bufs=1)
                        z1 = patn.tile([DH + 1, 512], F32, tag="z1", bufs=1)
                        qs = (sb * SBW, (sb + 1) * SBW)
                        for j in range(nkt):
                            # columns q < j*128 of this key tile are fully
                            # masked; restrict S/exp/PV to the valid suffix.
                            j_rel = j - 4 * sb
                            off = max(j_rel, 0) * 128
                            sp = patn.tile([128, 1024], F32, tag="sp", bufs=2)
                            ks = (j * 128, (j + 1) * 128)
                            nc.tensor.matmul(
                                sp[:, off:512],
                                kt[0:64, ks[0]:ks[1]],
                                qt[0:64, qs[0] + off:qs[1]],
                                start=True, stop=True,
                                tile_position=(0, 0),
                            )
                            nc.tensor.matmul(
                                sp[:, 512 + off:1024],
                                kt[64:128, ks[0]:ks[1]],
                                qt[64:128, qs[0] + off:qs[1]],
                                start=True, stop=True,
                                tile_position=(64, 0),
                            )
                            if j_rel >= 0:
                                for u in (0, 1):
                                    lo = u * 512 + off
                                    nc.vector.tensor_add(
                                        sp[:, lo:lo + 128], sp[:, lo:lo + 128], msk_t[:]
                                    )
                            pt = ph12.tile([128, 1024], F32R, tag="pt", bufs=2)
                            sp3 = sp[:].rearrange("p (u q) -> p u q", u=2)
                            pt3 = pt[:].rearrange("p (u q) -> p u q", u=2)
                            nc.scalar.activation(
                                pt3[:, :, off:512], sp3[:, :, off:512], EXPF, scale=SCALE
                            )
                            nc.tensor.matmul(
                                z0[:, off:512],
                                v_t[j][:, 2 * p, :],
                                pt[:, off:512],
                                start=(j == 0), stop=(j == nkt - 1),
                            )
                            nc.tensor.matmul(
                                z1[:, off:512],
                                v_t[j][:, 2 * p + 1, :],
                                pt[:, 512 + off:1024],
                                start=(j == 0), stop=(j == nkt - 1),
                            )
                        # normalize by softmax denominator (row DH of z psum):
                        # pull the denom row to SBUF (same partition), invert it
                        # in place, broadcast-DMA across 64 partitions, multiply.
                        d0row = ph12.tile([1, 512], F32, tag="d0row", bufs=2)
                        d1row = ph12.tile([1, 512], F32, tag="d1row", bufs=2)
                        nc.vector.tensor_copy(d0row[:], z0[DH:DH + 1, :])
                        nc.vector.tensor_copy(d1row[:], z1[DH:DH + 1, :])
                        nc.vector.reciprocal_approx_fast(d0row[:], d0row[:])
                        nc.vector.reciprocal_approx_fast(d1row[:], d1row[:])
                        r0 = ph12.tile([64, 512], F32, tag="r0", bufs=1)
                        r1 = ph12.tile([64, 512], F32, tag="r1", bufs=1)
                        nc.gpsimd.partition_broadcast(r0[:], d0row[:], channels=64)
                        nc.gpsimd.partition_broadcast(r1[:], d1row[:], channels=64)
                        nc.vector.tensor_mul(
                            z_t[p][0:64, qs[0]:qs[1]], z0[0:64, :], r0[:]
                        )
                        t1 = ph12.tile([64, 512], F32R, tag="t1", bufs=1)
                        nc.vector.tensor_mul(t1[:], z1[0:64, :], r1[:])
                        nc.sync.dma_start(z_t[p][64:128, qs[0]:qs[1]], t1[:])

            # ---- phase O: out[q, m] = sum_p z_p^T.T @ wo_p
            with (
                tc.tile_pool(name="ph3", bufs=1) as ph3,
                tc.tile_pool(name="po", bufs=1, space="PSUM") as po,
            ):
                wo_t = [ph3.tile([128, DM], F32R, tag=f"wo{p}", name=f"wo{p}") for p in range(PAIRS)]
                for p in range(PAIRS):
                    nc.sync.dma_start(wo_t[p][:], wo_d.ap()[p])
                for q in range(NST):
                    for mb in range(2):
                        ps = po.tile([128, 512], F32, tag="o", bufs=4)
                        for p in range(PAIRS):
                            nc.tensor.matmul(
                                ps[:],
                                z_t[p][:, q * 128:(q + 1) * 128],
                                wo_t[p][:, mb * 512:(mb + 1) * 512],
                                start=(p == 0),
                                stop=(p == PAIRS - 1),
                            )
                        ost = ph3.tile([128, 512], F32, tag="ost", bufs=4)
                        nc.vector.tensor_copy(ost[:], ps[:])
                        nc.sync.dma_start(
                            out_d.ap()[q * 128:(q + 1) * 128, mb * 512:(mb + 1) * 512],
                            ost[:],
                        )

    nc.compile()
    return nc


def _get_nc():
    if "nc" not in _NC_CACHE:
        _NC_CACHE["nc"] = _build_nc()
    return _NC_CACHE["nc"]


def _causal_masks():
    k = np.arange(128)[:, None]
    q = np.arange(128)[None, :]
    return np.where(q >= k, 0.0, MASK_NEG).astype(np.float32)


def kernel(resid_pre, W_Q, W_K, W_V, W_O, b_Q, b_K, b_V, b_O):
    global LAST_RESULTS
    resid_pre = np.asarray(resid_pre, dtype=np.float32)
    W_Q = np.asarray(W_Q, dtype=np.float32)
    W_K = np.asarray(W_K, dtype=np.float32)
    W_V = np.asarray(W_V, dtype=np.float32)
    W_O = np.asarray(W_O, dtype=np.float32)
    b_Q = np.asarray(b_Q, dtype=np.float32)
    b_K = np.asarray(b_K, dtype=np.float32)
    b_V = np.asarray(b_V, dtype=np.float32)
    b_O = np.asarray(b_O, dtype=np.float32)

    B = resid_pre.shape[0]
    masks = _causal_masks()

    def pack_pairs(w):  # [8, 1024, 64] -> [4, 8, 128, 128]
        return np.ascontiguousarray(
            w.reshape(PAIRS, 2, DM, DH).transpose(0, 2, 1, 3).reshape(PAIRS, MC, 128, 128)
        )

    in_maps = []
    for c in range(8):
        b, g = divmod(c, 2)
        hs = slice(g * NHC, (g + 1) * NHC)
        in_maps.append({
            "xt": np.ascontiguousarray(resid_pre[b].T),
            "wq": pack_pairs(W_Q[hs]),
            "wk": pack_pairs(W_K[hs]),
            "wv": np.ascontiguousarray(
                W_V[hs].transpose(1, 0, 2).reshape(DM, NHC * DH).reshape(MC, 128, NHC * DH)
            ),
            "wo": np.ascontiguousarray(W_O[hs].reshape(PAIRS, 128, DM)),
            "bq": np.ascontiguousarray(b_Q[hs].reshape(PAIRS, 128, 1)),
            "bk": np.ascontiguousarray(b_K[hs].reshape(PAIRS, 128, 1)),
            "mask": masks,
        })

    nc = _get_nc()
    res = bass_utils.run_bass_kernel_spmd(nc, in_maps, core_ids=list(range(8)))
    LAST_RESULTS = res

    # b_V contributes exactly sum_h W_O[h].T @ b_V[h] (softmax rows sum to 1)
    const = np.einsum("hdm,hd->m", W_O, b_V).astype(np.float32) + b_O
    out = np.empty((B, S, DM), dtype=np.float32)
    for b in range(B):
        out[b] = res.results[2 * b]["out"] + res.results[2 * b + 1]["out"] + const
    return out
